# revision 3
# baseline (speedup 1.0000x reference)
"""Trainium2 Bass kernel for a single ALiBi causal attention head.

reference:
    k = x @ Wk; q = x @ Wq; v = x @ Wv          (x: [B,T,E], W*: [E,H])
    scores = q k^T / sqrt(H) + m*(j-i)
    causal mask (if is_decoder), softmax, y = attn @ v
    returns (y [B,T,H], attn [B,T,T])

Strategy: data-parallel over batch B=8 across the 8 NeuronCores. Each core
computes one batch element end-to-end; no collectives. Host passes x
pre-transposed ([E,T]) so the E-contraction lands on SBUF partitions, and
the ALiBi bias is folded into the QK^T matmul via two extra contraction
rows (ramp vectors). Scores are computed twice (natural layout for the attn
output; transposed layout for the attn@v matmul) because recomputing the
cheap QK^T matmul is ~10x cheaper than transposing the 16MB attn matrix.
"""

import sys

for _p in ("/opt/trn_rl_repo",):
    if _p not in sys.path:
        sys.path.append(_p)

import numpy as np

import concourse.bass as bass
import concourse.mybir as mybir
import concourse.tile as tile
from concourse import bacc
from concourse.bass_utils import run_bass_kernel_spmd

N_CORES = 8
M_SLOPE = 0.0625
F32 = mybir.dt.float32
F32R = mybir.dt.float32r


def build_graph(T, E, H, is_decoder, use_f32r=True):
    """Build the single-core graph (SPMD-replicated across 8 cores)."""
    assert T % 512 == 0 and E % 128 == 0
    QT = 128          # q rows per phase-2 tile
    KT = 512          # k cols per phase-2 tile (max fp32 moving dim)
    JT = 128          # k rows per phase-3 tile
    QB = 512          # q cols per phase-3 block
    EC = E // 128     # contraction chunks
    n_qt = T // QT
    n_kt = T // KT
    n_jt = T // JT
    n_qb = T // QB
    HA = H + 2        # augmented contraction depth for QK^T (bias rows)

    # dtype for every tensor consumed by a (non-transpose) matmul: the BIR
    # verifier requires f32r matmul operands to be *produced* as f32r, so the
    # whole chain (DRAM param -> DMA -> SBUF tile / PSUM-copy out) is typed
    # f32r; numpy side stays float32 (same 4-byte layout).
    MMD = F32R if use_f32r else F32

    nc = bacc.Bacc("TRN2", target_bir_lowering=False, debug=False,
                   num_devices=N_CORES)

    xT = nc.dram_tensor("xT", [E, T], MMD, kind="ExternalInput").ap()
    wq = nc.dram_tensor("wq", [E, H], MMD, kind="ExternalInput").ap()
    wk = nc.dram_tensor("wk", [E, H], MMD, kind="ExternalInput").ap()
    wv = nc.dram_tensor("wv", [E, H], MMD, kind="ExternalInput").ap()
    rq = nc.dram_tensor("rq", [2, T], MMD, kind="ExternalInput").ap()
    rk = nc.dram_tensor("rk", [2, T], MMD, kind="ExternalInput").ap()
    attn = nc.dram_tensor("attn", [T, T], F32, kind="ExternalOutput").ap()
    y = nc.dram_tensor("y", [T, H], F32, kind="ExternalOutput").ap()

    with tile.TileContext(nc) as tc, tc.tile_pool(name="singles", bufs=1) as singles, \
            tc.tile_pool(name="xc", bufs=EC) as xc_pool, \
            tc.tile_pool(name="ps", bufs=4, space="PSUM") as ps_pool, \
            tc.tile_pool(name="yu_ps", bufs=2, space="PSUM") as yu_pool, \
            tc.tile_pool(name="yt_ps", bufs=2, space="PSUM") as yt_pool, \
            tc.tile_pool(name="exprow", bufs=2) as exprow_pool, \
            tc.tile_pool(name="attnrow", bufs=2) as attnrow_pool, \
            tc.tile_pool(name="sc", bufs=3) as sc_pool, \
            tc.tile_pool(name="expt", bufs=3) as expt_pool, \
            tc.tile_pool(name="small", bufs=8) as small_pool:

        # ---- constants ----
        ident = singles.tile([128, 128], F32)
        nc.gpsimd.memset(ident, 0.0)
        nc.gpsimd.affine_select(
            out=ident, in_=ident, compare_op=mybir.AluOpType.not_equal,
            fill=1.0, base=0, pattern=[[-1, 128]], channel_multiplier=1)

        if is_decoder:
            # additive causal masks, 4 diagonal offsets each for the natural
            # ([QT x KT], partition=i) and transposed ([JT x QB], partition=j)
            # score layouts
            masks_n = singles.tile([128, 4, KT], F32)
            nc.gpsimd.memset(masks_n, 0.0)
            for v in range(4):
                # keep (0.0) where jc <= ic + 128*v, else -1e9
                nc.gpsimd.affine_select(
                    out=masks_n[:, v, :], in_=masks_n[:, v, :],
                    compare_op=mybir.AluOpType.is_ge, fill=-1e9,
                    base=128 * v, pattern=[[-1, KT]], channel_multiplier=1)
            masks_t = singles.tile([128, 4, QB], F32)
            nc.gpsimd.memset(masks_t, 0.0)
            for v in range(4):
                # keep where ic - jc - 128*v >= 0 (partition = jc, free = ic)
                nc.gpsimd.affine_select(
                    out=masks_t[:, v, :], in_=masks_t[:, v, :],
                    compare_op=mybir.AluOpType.is_ge, fill=-1e9,
                    base=-128 * v, pattern=[[1, QB]], channel_multiplier=-1)

        # ---- weights + ramps + x ----
        w_sb = singles.tile([128, 3, EC, H], MMD)
        for i, w in enumerate((wq, wk, wv)):
            nc.sync.dma_start(
                out=w_sb[:, i, :, :],
                in_=w.rearrange("(c p) h -> p c h", p=128))

        qa = singles.tile([HA, T], MMD)
        ka = singles.tile([HA, T], MMD)
        vt = singles.tile([H, T], F32)
        nc.sync.dma_start(out=qa[H:HA, :], in_=rq)
        nc.sync.dma_start(out=ka[H:HA, :], in_=rk)

        x_sb = []
        for c in range(EC):
            xt = xc_pool.tile([128, T], MMD, tag="xchunk")
            nc.sync.dma_start(out=xt, in_=xT[c * 128:(c + 1) * 128, :])
            x_sb.append(xt)

        # ---- phase 1: q^T / k^T / v^T projections ----
        for t4 in range(T // 512):
            tsl = slice(t4 * 512, (t4 + 1) * 512)
            for i, dest in enumerate((qa, ka, vt)):
                ps = ps_pool.tile([H, 512], F32, tag="ps")
                for c in range(EC):
                    nc.tensor.matmul(
                        ps, w_sb[:, i, c, :], x_sb[c][:, tsl],
                        start=(c == 0), stop=(c == EC - 1))
                nc.vector.tensor_copy(dest[0:H, tsl], ps)

        # ---- phase 1.5: v natural tiles with ones column ----
        vaug = singles.tile([128, n_jt, H + 1], MMD)
        nc.vector.memset(vaug[:, :, H:H + 1].bitcast(F32), 1.0)
        for jt in range(n_jt):
            pt = yt_pool.tile([128, H + 1], F32, tag="yt")
            nc.tensor.transpose(
                pt[:, 0:H], vt[:, jt * 128:(jt + 1) * 128], ident[0:H, 0:H])
            nc.scalar.copy(vaug[:, jt, 0:H], pt[:, 0:H])

        y_sb = singles.tile([128, n_qt, H], F32)

        def phase2(qi):
            q0 = qi * QT
            bmax = (q0 // KT + 1) if is_decoder else n_kt
            valid = bmax * KT
            exp_row = exprow_pool.tile([QT, T], F32, tag="exprow")
            zpart = small_pool.tile([QT, n_kt], F32, tag="zpart")
            for b in range(bmax):
                ps = ps_pool.tile([QT, KT], F32, tag="ps")
                nc.tensor.matmul(
                    ps, qa[:, q0:q0 + QT],
                    ka[:, b * KT:(b + 1) * KT], start=True, stop=True)
                src = ps
                if is_decoder and b == bmax - 1:
                    off = q0 - b * KT
                    sc = sc_pool.tile([QT, KT], F32, tag="sc")
                    nc.vector.tensor_add(sc, ps, masks_n[:, off // 128, :])
                    src = sc
                nc.scalar.activation(
                    exp_row[:, b * KT:(b + 1) * KT], src,
                    mybir.ActivationFunctionType.Exp,
                    accum_out=zpart[:, b:b + 1])
            z = small_pool.tile([QT, 1], F32, tag="z")
            nc.vector.reduce_sum(z, zpart[:, 0:bmax], axis=mybir.AxisListType.X)
            zr = small_pool.tile([QT, 1], F32, tag="zr")
            nc.vector.reciprocal(zr, z)
            attn_row = attnrow_pool.tile([QT, T], F32, tag="attnrow")
            nc.vector.tensor_scalar_mul(
                attn_row[:, 0:valid], exp_row[:, 0:valid], zr)
            nc.sync.dma_start(
                out=attn[q0:q0 + QT, 0:valid], in_=attn_row[:, 0:valid])
            # above-diagonal tail stays zero: output buffers are pre-zeroed

        def phase3(qb):
            q0 = qb * QB
            jmax = (4 * (qb + 1)) if is_decoder else n_jt
            yps = yu_pool.tile([H + 1, QB], F32, tag="yu")
            for jt in range(jmax):
                k0 = jt * JT
                ps = ps_pool.tile([JT, QB], F32, tag="ps")
                nc.tensor.matmul(
                    ps, ka[:, k0:k0 + JT], qa[:, q0:q0 + QB],
                    start=True, stop=True)
                src = ps
                if is_decoder and k0 >= q0:
                    off = k0 - q0
                    sc = sc_pool.tile([JT, QB], F32, tag="sc")
                    nc.vector.tensor_add(sc, ps, masks_t[:, off // 128, :])
                    src = sc
                expt = expt_pool.tile([JT, QB], MMD, tag="expt")
                nc.scalar.activation(
                    expt, src, mybir.ActivationFunctionType.Exp)
                nc.tensor.matmul(
                    yps, vaug[:, jt, :], expt,
                    start=(jt == 0), stop=(jt == jmax - 1))
            yu_sb = sc_pool.tile([H + 1, QB], F32, tag="yusb")
            nc.scalar.copy(yu_sb, yps)
            for s in range(4):
                qi = qb * 4 + s
                pt = yt_pool.tile([128, H + 1], F32, tag="yt")
                nc.tensor.transpose(
                    pt, yu_sb[:, s * 128:(s + 1) * 128],
                    ident[0:H + 1, 0:H + 1])
                zr3 = small_pool.tile([128, 1], F32, tag="zr3")
                nc.vector.reciprocal(zr3, pt[:, H:H + 1])
                nc.vector.tensor_scalar_mul(
                    y_sb[:, qi, :], pt[:, 0:H], zr3)

        for qb in range(n_qb):
            for s in range(4):
                phase2(qb * 4 + s)
            phase3(qb)

        nc.sync.dma_start(
            out=y.rearrange("(t p) h -> p t h", p=128), in_=y_sb)

    nc.compile()
    return nc


_GRAPH_CACHE = {}


def _get_graph(T, E, H, is_decoder):
    key = (T, E, H, bool(is_decoder))
    if key not in _GRAPH_CACHE:
        _GRAPH_CACHE[key] = build_graph(T, E, H, bool(is_decoder))
    return _GRAPH_CACHE[key]


def kernel(x, Wk, Wq, Wv, is_decoder):
    x = np.asarray(x, dtype=np.float32)
    Wk = np.asarray(Wk, dtype=np.float32)
    Wq = np.asarray(Wq, dtype=np.float32)
    Wv = np.asarray(Wv, dtype=np.float32)
    dec = bool(int(is_decoder))
    B, T, E = x.shape
    H = Wk.shape[1]
    assert B == N_CORES

    m = np.float32(M_SLOPE)
    i_idx = np.arange(T, dtype=np.float32)
    if dec:
        rq_np = np.stack([-m * i_idx, np.ones(T, np.float32)])
    else:
        # drop the per-row -m*i term (softmax-invariant) and shift globally
        # so exp never overflows without causal truncation
        rq_np = np.stack([np.full(T, -m * (T - 1), np.float32),
                          np.ones(T, np.float32)])
    rk_np = np.stack([np.ones(T, np.float32), m * i_idx])

    wq_s = np.ascontiguousarray(Wq * np.float32(H ** -0.5))
    in_maps = []
    for b in range(B):
        in_maps.append({
            "xT": np.ascontiguousarray(x[b].T),
            "wq": wq_s,
            "wk": np.ascontiguousarray(Wk),
            "wv": np.ascontiguousarray(Wv),
            "rq": np.ascontiguousarray(rq_np),
            "rk": np.ascontiguousarray(rk_np),
        })

    nc = _get_graph(T, E, H, dec)
    res = run_bass_kernel_spmd(nc, in_maps, core_ids=list(range(N_CORES)))
    y = np.stack([res.results[b]["y"] for b in range(B)])
    attn = np.stack([res.results[b]["attn"] for b in range(B)])
    return (y, attn)
